# revision 29
# baseline (speedup 1.0000x reference)
"""Multi-head self-attention (B=4, L=2048, D=1024, H=16) on 8 trn2 cores.

Sharding: DP=4 over batch x TP=2 over heads (8 heads/core). Each core:
  QT/KT = W.T @ x.T projections (c on partitions), V natural layout,
  S^T = K Q^T per head with keys on partitions, exp on ACT, causal via
  tile skipping + gpsimd affine_select on diagonal tiles, ctx^T =
  V'.T @ P^T with a ones-column in V' producing the softmax denominator
  row.  Normalization: DVE reciprocal_approx_fast on the denominator
  row, gpsimd partition_broadcast to 64 partitions, one DVE multiply.
  V bias is folded into V' before attention (bv is invariant under the
  softmax average).  Output projection DMAs straight from PSUM; host
  sums the TP pair partials + b_lin.
"""

import sys

if "/opt/trn_rl_repo" not in sys.path:
    sys.path.insert(0, "/opt/trn_rl_repo")

import numpy as np

import concourse.bass as bass
import concourse.tile as tile
from concourse import bacc, mybir
from concourse.bass_utils import run_bass_kernel_spmd

B, L, D = 4, 2048, 1024
H, HD = 16, 64
HPC = H // 2          # heads per core (TP=2)
CPC = HPC * HD        # head-dim columns per core = 512
N_CORES = 8
PP_BUFS = 4
BIG_BUFS = 2

F32 = mybir.dt.float32
F16 = mybir.dt.float16

NB = 4                # 512-wide i/l blocks
BW = L // NB          # 512
NT = L // 128         # 16 j/l tiles of 128
DT = D // 128         # 8 d-tiles
CT = CPC // 128       # 4 c-tiles per core


def build(tc, io):
    nc = tc.nc
    xbT = io["xbT"].rearrange("(t p) l -> t p l", p=128)      # [8,128,2048] f16
    wq = io["wq"].rearrange("(t p) c -> p t c", p=128)        # [128,8,512] view
    wk = io["wk"].rearrange("(t p) c -> p t c", p=128)
    wv = io["wv"].rearrange("(t p) c -> p t c", p=128)
    wo = io["wo"].rearrange("(t p) e -> p t e", p=128)        # [128,4,1024]
    bq = io["bq"].rearrange("(t p) -> p t", p=128)            # [128,4] f32
    bk = io["bk"].rearrange("(t p) -> p t", p=128)
    out = io["out"].rearrange("(t p) e -> t p e", p=128)      # [16,128,1024] f32

    singles = tc.alloc_tile_pool(name="singles", bufs=1)
    xpool = tc.alloc_tile_pool(name="xpool", bufs=16)
    work = tc.alloc_tile_pool(name="work", bufs=4)
    psum = tc.alloc_tile_pool(name="psum", bufs=4, space="PSUM")

    # --- x block 0 first: it gates the very first projection matmul ---------
    xts = {}

    def load_x(blk):
        xt = [xpool.tile([128, BW], F16, tag="xbt", name=f"xt{blk}_{dt}")
              for dt in range(DT)]
        for dt in range(DT):
            nc.sync.dma_start(out=xt[dt], in_=xbT[dt, :, blk * BW : (blk + 1) * BW])
        xts[blk] = xt

    # --- resident tensors; first projection needs only (xt dt, wq dt) pairs,
    # so interleave those DMAs per d-tile to minimize time-to-first-matmul.
    wq_sb = singles.tile([128, DT, CPC], F16)
    wk_sb = singles.tile([128, DT, CPC], F16)
    wv_sb = singles.tile([128, DT, CPC], F16)
    wo_sb = singles.tile([128, CT, D], F16)
    bq_sb = singles.tile([128, CT], F32)
    bk_sb = singles.tile([128, CT], F32)
    bv_row = singles.tile([1, CPC], F32)
    bv128 = singles.tile([128, CPC], F32)
    nc.sync.dma_start(out=bq_sb, in_=bq)
    xt0 = [xpool.tile([128, BW], F16, tag="xbt", name=f"xt0_{dt}")
           for dt in range(DT)]
    for dt in range(DT):
        nc.sync.dma_start(out=xt0[dt], in_=xbT[dt, :, 0:BW])
        nc.sync.dma_start(out=wq_sb[:, dt, :], in_=wq[:, dt, :])
    xts = {0: xt0}
    nc.sync.dma_start(out=bk_sb, in_=bk)
    for dt in range(DT):
        nc.sync.dma_start(out=wk_sb[:, dt, :], in_=wk[:, dt, :])
    nc.sync.dma_start(out=wv_sb, in_=wv)
    nc.sync.dma_start(out=bv_row, in_=io["bvr"].rearrange("(o c) -> o c", o=1))
    nc.gpsimd.partition_broadcast(bv128, bv_row)
    bv128v = bv128.rearrange("p (h d) -> p h d", h=HPC)

    qt_sb = singles.tile([128, CT, L], F16)    # Q^T  [c, i]
    kt_sb = singles.tile([128, CT, L], F16)    # K^T  [c, j]
    vp_sb = singles.tile([128, NT, HPC, HD + 1], F16)   # V' [j, (h, d'|1)]
    cxt_sb = singles.tile([128, CT, L], F16)   # normalized ctx^T [d', i]
    nc.gpsimd.memset(vp_sb[:, :, :, HD : HD + 1], 1.0)

    def proj(blk):
        xt = xts[blk]
        for w_sb, b_sb, t_sb, nm in ((wq_sb, bq_sb, qt_sb, "q"),
                                     (wk_sb, bk_sb, kt_sb, "k")):
            for cp in range(2):
                ps_p = [psum.tile([128, BW], F32, tag="pp", bufs=PP_BUFS,
                                  name=f"ps{nm}{blk}{cp}_{ci}") for ci in range(2)]
                for dt in range(DT):
                    for ci in range(2):
                        nc.tensor.matmul(
                            ps_p[ci],
                            lhsT=w_sb[:, dt, (2 * cp + ci) * 128 : (2 * cp + ci + 1) * 128],
                            rhs=xt[dt], start=(dt == 0), stop=(dt == DT - 1))
                for ci in range(2):
                    ct = 2 * cp + ci
                    nc.vector.tensor_scalar_add(
                        out=t_sb[:, ct, blk * BW : (blk + 1) * BW],
                        in0=ps_p[ci], scalar1=b_sb[:, ct : ct + 1])
        for j in range(4):
            lt = 4 * blk + j
            ps_v = psum.tile([128, CPC], F32, tag="pp", bufs=PP_BUFS,
                             name=f"psv{lt}")
            for dt in range(DT):
                nc.tensor.matmul(
                    ps_v, lhsT=xt[dt][:, j * 128 : (j + 1) * 128],
                    rhs=wv_sb[:, dt, :], start=(dt == 0), stop=(dt == DT - 1))
            # fold bv into V': softmax-averaging ones gives +bv exactly.
            nc.vector.tensor_tensor(
                out=vp_sb[:, lt, :, 0:HD],
                in0=ps_v.rearrange("p (h d) -> p h d", h=HPC),
                in1=bv128v, op=mybir.AluOpType.add)

    def attn_gen(hp, ib):
        ctx_ps = [psum.tile([128, BW], F32, tag="pp", bufs=PP_BUFS,
                            name=f"cx{hp}_{ib}_{hh}") for hh in range(2)]
        ntj = 4 * ib + 4

        def scores(tj):
            k = tj - 4 * ib
            off = 128 * k if k >= 0 else 0
            w = BW - off
            st = psum.tile([128, 2, BW], F32, tag="big", bufs=BIG_BUFS,
                           name=f"st{hp}_{ib}_{tj}")
            # bufs=6: with two interleaved iterations sharing this tag, a
            # 4-deep rotation creates a WAR cycle in the PE queue.
            pt = work.tile([128, 2, BW], F16, tag="pt", bufs=6,
                           name=f"pt{hp}_{ib}_{tj}")
            for hh in range(2):
                nc.tensor.matmul(
                    st[:, hh, off:], skip_group_check=True, start=True, stop=True,
                    lhsT=kt_sb[64 * hh : 64 * hh + 64, hp,
                               tj * 128 : (tj + 1) * 128],
                    rhs=qt_sb[64 * hh : 64 * hh + 64, hp,
                              ib * BW + off : (ib + 1) * BW])
            nc.scalar.activation(
                out=pt[:, :, off:], in_=st[:, :, off:],
                func=mybir.ActivationFunctionType.Exp, scale=0.125)
            if k >= 0:
                # zero the j>i half of the diagonal tile in place
                nc.gpsimd.affine_select(
                    out=pt[:, :, off:], in_=pt[:, :, off:],
                    compare_op=mybir.AluOpType.is_ge, fill=0.0,
                    base=0, pattern=[[0, 2], [1, w]], channel_multiplier=-1)
            return tj, off, pt

        def ctx(tj, off, pt):
            for hh in range(2):
                nc.tensor.matmul(
                    ctx_ps[hh][0 : HD + 1, off:], skip_group_check=True,
                    lhsT=vp_sb[:, tj, 2 * hp + hh, :],
                    rhs=pt[:, hh, off:], start=(tj == 0), stop=(tj == ntj - 1))

        # software pipeline: scores(tj+1) issued before ctx(tj) so the PE
        # queue alternates [st pair | ctx pair] with 1-tile lookahead.
        pend = None
        for tj in range(ntj):
            cur = scores(tj)
            if pend is not None:
                ctx(*pend)
            pend = cur
            yield
        ctx(*pend)
        rdens = []
        for hh in range(2):
            # reciprocal_approx_fast misreads non-zero partition bases;
            # stage the denominator row at partition 0 first.
            den = work.tile([1, BW], F32, tag="den", name=f"dn{hp}{ib}{hh}")
            nc.vector.tensor_copy(out=den, in_=ctx_ps[hh][HD : HD + 1, :])
            rden = work.tile([1, BW], F32, tag="rden", name=f"rd{hp}{ib}{hh}")
            nc.vector.reciprocal_approx_fast(out=rden, in_=den)
            rdens.append(rden)
        rbs = []
        for hh in range(2):
            rb = work.tile([64, BW], F32, tag="rb", name=f"rb{hp}{ib}{hh}")
            nc.gpsimd.partition_broadcast(rb, rdens[hh])
            rbs.append(rb)
        for hh in range(2):
            nc.vector.tensor_tensor(
                out=cxt_sb[64 * hh : 64 * hh + 64, hp, ib * BW : (ib + 1) * BW],
                in0=ctx_ps[hh][0:HD, :], in1=rbs[hh],
                op=mybir.AluOpType.mult)

    def outproj(it):
        ps_o = [psum.tile([128, 512], F32, tag="pp", bufs=PP_BUFS,
                          name=f"po{it}_{eb}") for eb in range(2)]
        for ct in range(CT):
            for eb in range(2):
                nc.tensor.matmul(
                    ps_o[eb],
                    lhsT=cxt_sb[:, ct, it * 128 : (it + 1) * 128],
                    rhs=wo_sb[:, ct, eb * 512 : (eb + 1) * 512],
                    start=(ct == 0), stop=(ct == CT - 1))
        o_sb = work.tile([128, D], F32, tag="osb", name=f"os{it}")
        # split evacuation across ACT and DVE (both idle here) so the two
        # halves copy in parallel and the PSUM pair frees sooner
        nc.scalar.copy(out=o_sb[:, 0:512], in_=ps_o[0])
        nc.sync.dma_start(out=out[it][:, 0:512], in_=o_sb[:, 0:512])
        nc.vector.tensor_copy(out=o_sb[:, 512:1024], in_=ps_o[1])
        nc.sync.dma_start(out=out[it][:, 512:1024], in_=o_sb[:, 512:1024])

    for blk in range(NB):
        if blk + 1 < NB:
            load_x(blk + 1)
        proj(blk)
    nc.sync.dma_start(out=wo_sb, in_=wo)   # needed only by outproj
    # Pre-roll: during each iteration's LAST four tile-steps, interleave the
    # successor's first steps. Every boundary's warm-up and normalization
    # then overlaps a dense exp/matmul stream, without disturbing the dense
    # middle of big iterations (which a uniform trickle measurably hurts).
    gens = [(attn_gen(hp, ib), 4 * ib + 4)
            for hp in range(CT) for ib in range(NB)]
    _S = object()
    carry = 0
    for idx, (g, ntj) in enumerate(gens):
        nxt = gens[idx + 1][0] if idx + 1 < len(gens) else None
        pre = 0
        for i in range(carry, ntj):
            next(g, _S)                      # one tile-step of this iteration
            if nxt is not None and i >= ntj - 6:
                next(nxt, _S)                # pre-roll one successor step
                pre += 1
        next(g, _S)                          # tail: last ctx + normalization
        carry = pre
    for it in range(NT):
        outproj(it)

    for p in (psum, work, xpool, singles):
        p.release()


_CACHE = {}


def _compiled():
    if "nc" in _CACHE:
        return _CACHE["nc"]
    nc = bacc.Bacc("TRN2", target_bir_lowering=False, debug=False)
    io = {
        "xbT": nc.dram_tensor("xbT", [D, L], F16, kind="ExternalInput").ap(),
        "wq": nc.dram_tensor("wq", [D, CPC], F16, kind="ExternalInput").ap(),
        "wk": nc.dram_tensor("wk", [D, CPC], F16, kind="ExternalInput").ap(),
        "wv": nc.dram_tensor("wv", [D, CPC], F16, kind="ExternalInput").ap(),
        "wo": nc.dram_tensor("wo", [CPC, D], F16, kind="ExternalInput").ap(),
        "bq": nc.dram_tensor("bq", [CPC], F32, kind="ExternalInput").ap(),
        "bk": nc.dram_tensor("bk", [CPC], F32, kind="ExternalInput").ap(),
        "bvr": nc.dram_tensor("bvr", [CPC], F32, kind="ExternalInput").ap(),
        "out": nc.dram_tensor("out", [L, D], F32, kind="ExternalOutput").ap(),
    }
    with tile.TileContext(nc) as tc:
        build(tc, io)
    nc.compile()
    _CACHE["nc"] = nc
    return nc


def make_in_maps(x, W_qkv, b_qkv, W_lin):
    f16 = mybir.dt.np(F16)
    in_maps = []
    for c in range(N_CORES):
        b, g = divmod(c, 2)
        cs = slice(CPC * g, CPC * (g + 1))
        in_maps.append({
            "xbT": np.ascontiguousarray(x[b].T).astype(f16),
            "wq": W_qkv[:, cs].astype(f16),
            "wk": W_qkv[:, D + CPC * g : D + CPC * (g + 1)].astype(f16),
            "wv": W_qkv[:, 2 * D + CPC * g : 2 * D + CPC * (g + 1)].astype(f16),
            "wo": W_lin[cs, :].astype(f16),
            "bq": b_qkv[cs].astype(np.float32),
            "bk": b_qkv[D + CPC * g : D + CPC * (g + 1)].astype(np.float32),
            "bvr": b_qkv[2 * D + CPC * g : 2 * D + CPC * (g + 1)].astype(np.float32),
        })
    return in_maps


def kernel(x, W_qkv, b_qkv, W_lin, b_lin, _trace=False):
    nc = _compiled()
    in_maps = make_in_maps(x, W_qkv, b_qkv, W_lin)
    res = run_bass_kernel_spmd(nc, in_maps, core_ids=list(range(N_CORES)),
                               trace=_trace)
    parts = [r["out"] for r in res.results]
    out = np.empty((B, L, D), dtype=np.float32)
    for b in range(B):
        out[b] = parts[2 * b] + parts[2 * b + 1] + b_lin.astype(np.float32)
    if _trace:
        return out, res
    return out
